# revision 16
# baseline (speedup 1.0000x reference)
"""KappaGCN (Poincare ball, K=-1) on 8 Trainium2 NeuronCores.

Sharding: rows of A / X over cores (1024 nodes/core). Layout choices that
carry the speed:
- The host ships each core its A slice PRE-TRANSPOSED (A^T[:, rows_i]),
  scaled by 8192 and cast to fp8-e4m3 (TRN e4 max-normal 240 >> 2.4 max),
  laid out exactly as the resident SBUF tile [128, 64 k-tiles, 1024] - zero
  on-device transposes of A, one 8 MB HBM read serving all three A@(.)
  passes (fp16/fp8 matmuls at 1 cycle/row, fp32 PSUM accumulation).
- Every scalar-engine activation comes from the single natural_log_exp
  table set: sqrt(x)=exp(0.5 ln x), rsqrt free via exp(-0.5 ln x),
  tanh(t)=1-2/(e^2t+1) - exactly one ACT_TABLE_LOAD in the whole kernel.
- Per-node math is batched across node blocks ([128, blocks] tiles), with
  norms via one Square + one segmented vector reduce.
- Boundaries are split into half-batches pipelined against the pass
  512-column PSUM chunks; G2/logits halves AllGather (fp16) as soon as
  each half is ready so gathers overlap the other half's compute.
- A gpsimd marker copy holds the 8 MB A^T stream until the small phase-0
  inputs land, so the DMA firehose cannot starve the critical path.

Per core: G1 = f(X) -> AllGather -> C1^T = G1^T A_i^T -> H1 -> G2
          -> AllGather x2 (halves) -> C2^T -> H2 -> logits L
          -> AllGather x2 -> out^T = L^T A_i^T.
"""
import os
import sys
import numpy as np

os.environ.setdefault("NEURON_RT_RESET_CORES", "1")
os.environ.setdefault("MYCRO_LOCAL_CACHE", "1")

for _p in ("/opt/trn_rl_repo",):
    if _p not in sys.path:
        sys.path.insert(0, _p)

import concourse.bass as bass
import concourse.mybir as mybir
import concourse.tile as tile
from concourse.masks import make_identity
from concourse.bass_utils import run_bass_kernel_spmd

F32 = mybir.dt.float32
F16 = mybir.dt.float16
F8 = mybir.dt.float8e4
AF = mybir.ActivationFunctionType
ALU = mybir.AluOpType

N_FULL = 8192
D = 32
C = 16
NCORES = 8
GC = D + 2          # G columns: [gamma*Y (32) | gamma-1 | 0.5]
CLIP = 1.0 - 1e-7
EPS = 1e-15
EPS2 = 1e-30        # clamp on squared norms == EPS clamp on norms
SCALE_A = 8192.0    # A premultiplied by this on host (fp16 normal range)


def _split_multiwaits(nc, limit=1):
    """This toolchain's walrus rejects instructions with more than `limit`
    sync waits; peel excess waits onto standalone EventSemaphore carriers
    placed just before, on the same engine queue (order-preserving)."""
    n_new = 0
    for bb in nc.main_func.blocks:
        out = []
        changed = False
        for ins in bb.instructions:
            si = getattr(ins, "sync_info", None)
            waits = list(si.on_wait) if si is not None and si.on_wait else []
            if len(waits) > limit:
                changed = True
                excess, keep = waits[:-limit], waits[-limit:]
                for i in range(0, len(excess), limit):
                    n_new += 1
                    out.append(mybir.InstEventSemaphore(
                        name=f"mwsplit_{n_new}_{ins.name}",
                        engine=ins.engine,
                        ins=[], outs=[],
                        sync_info=mybir.SyncInfo(
                            on_wait=excess[i:i + limit], on_update=[]),
                    ))
                try:
                    si.on_wait = keep
                except Exception:
                    ins.sync_info = mybir.SyncInfo(
                        on_wait=keep, on_update=list(si.on_update))
            out.append(ins)
        if changed:
            try:
                bb.instructions[:] = out
            except Exception:
                bb.set_instructions(out)
    return n_new


def build_program(N=N_FULL, ncores=NCORES):
    rows = N // ncores          # nodes per core
    MB = rows // 128            # node blocks per core
    KT = N // 128               # contraction tiles
    NS = [(s, min(512, rows - s)) for s in range(0, rows, 512)]
    CH = 8                      # A^T DMA chunks
    KC = KT // CH

    nc = bass.Bass(num_devices=ncores)

    At = nc.dram_tensor("At", [128, KT, rows], F8, kind="ExternalInput")
    Xp = nc.dram_tensor("Xp", [128, MB, D], F32, kind="ExternalInput")
    Xt = nc.dram_tensor("Xt", [D, rows], F32, kind="ExternalInput")
    W1t = nc.dram_tensor("W1t", [D, D], F32, kind="ExternalInput")
    W2t = nc.dram_tensor("W2t", [D, D], F32, kind="ExternalInput")
    WLb = nc.dram_tensor("WLb", [D, C], F32, kind="ExternalInput")
    PTn = nc.dram_tensor("PTn", [D, C], F32, kind="ExternalInput")
    cXW = nc.dram_tensor("cXW", [1, C], F32, kind="ExternalInput")
    cBA = nc.dram_tensor("cBA", [1, C], F32, kind="ExternalInput")
    cLA = nc.dram_tensor("cLA", [1, C], F32, kind="ExternalInput")
    outT = nc.dram_tensor("outT", [C, rows], F32, kind="ExternalOutput")

    g1_loc = nc.dram_tensor("g1_loc", [128, MB, GC // 2], F16)
    g1_full = nc.dram_tensor("g1_full", [ncores, 128, MB, GC // 2], F16,
                             addr_space="Shared")
    g2_loc = {}
    g2_full = {}
    l_loc = {}
    l_full = {}
    for h in (0, 1):
        g2_loc[h] = nc.dram_tensor(f"g2_loc{h}", [128, HB, GC // 2], F16)
        g2_full[h] = nc.dram_tensor(f"g2_full{h}", [ncores, 128, HB, GC // 2],
                                    F16, addr_space="Shared")
        l_loc[h] = nc.dram_tensor(f"l_loc{h}", [128, HB, C], F16)
        l_full[h] = nc.dram_tensor(f"l_full{h}", [ncores, 128, HB, C], F16,
                                   addr_space="Shared")
    rg = [list(range(ncores))]

    with tile.TileContext(nc, num_cores=ncores) as tc:
        import contextlib
        with contextlib.ExitStack() as ctx:
            singles = ctx.enter_context(tc.tile_pool(name="singles", bufs=1))
            sc = ctx.enter_context(tc.tile_pool(name="sc", bufs=1))
            vec = ctx.enter_context(tc.tile_pool(name="vec", bufs=1))
            ps_c = ctx.enter_context(tc.tile_pool(name="ps_c", bufs=1,
                                                  space="PSUM"))
            ps_s = ctx.enter_context(tc.tile_pool(name="ps_s", bufs=1,
                                                  space="PSUM"))

            # ---- small loads (sync queue, issued first) ----
            ident = singles.tile([128, 128], F32)
            make_identity(nc, ident[:])
            w1t_sb = singles.tile([D, D], F32, tag="w1t")
            nc.sync.dma_start(out=w1t_sb[:], in_=W1t[:, :])
            w2t_sb = singles.tile([D, D], F32, tag="w2t")
            nc.sync.dma_start(out=w2t_sb[:], in_=W2t[:, :])
            wlb_sb = singles.tile([D, C], F32, tag="wlb")
            nc.sync.dma_start(out=wlb_sb[:], in_=WLb[:, :])
            ptn_sb = singles.tile([D, C], F32, tag="ptn")
            nc.sync.dma_start(out=ptn_sb[:], in_=PTn[:, :])

            def bcast(dram):
                t = singles.tile([128, C], F32, tag=dram.name)
                nc.sync.dma_start(out=t[:],
                                  in_=bass.AP(dram, 0, [[0, 128], [1, C]]))
                return t
            cxw_sb = bcast(cXW)
            cba_sb = bcast(cBA)
            cla_sb = bcast(cLA)

            x_sb = singles.tile([128, MB, D], F32, tag="x_sb")
            nc.sync.dma_start(out=x_sb[:], in_=Xp[:, :, :])
            xt_sb = singles.tile([D, rows], F32, tag="xt_sb")
            nc.sync.dma_start(out=xt_sb[:], in_=Xt[:, :])

            # ---- A^T streaming into resident SBUF (gpsimd queue) ----
            # The gpsimd marker copy blocks the gpsimd engine until the small
            # loads land, so the 16MB A^T stream can't starve them of DMA
            # bandwidth (phase0 needs x/xt/weights immediately).
            marker = singles.tile([1, 4], F32, tag="marker")
            nc.gpsimd.tensor_copy(marker[:], xt_sb[0:1, 0:4])
            nc.gpsimd.tensor_copy(marker[:], x_sb[0:1, 0, 0:4])
            nc.gpsimd.tensor_copy(marker[:], w1t_sb[0:1, 0:4])
            at_all = singles.tile([128, KT, rows], F8, tag="at_all")
            for cch in range(CH):
                nc.gpsimd.dma_start(
                    out=at_all[:, cch * KC:(cch + 1) * KC, :],
                    in_=At[:, cch * KC:(cch + 1) * KC, :])

            # Junk operands for PE warm-keeper matmuls (hold the HAM clock
            # gate at 8/8 through otherwise PE-idle stretches). M=128 so the
            # activity monitor sees a busy array.
            warm_l = singles.tile([128, 128], F16, tag="warm_l")
            nc.vector.memset(warm_l[:], 0.0)
            warm_r = singles.tile([128, 256], F16, tag="warm_r")
            nc.vector.memset(warm_r[:], 0.0)

            g1sb = singles.tile([128, KT, GC], F16, tag="g1sb")
            g2sb = singles.tile([128, KT, GC], F16, tag="g2sb")
            lsb = singles.tile([128, KT, C], F16, tag="lsb")

            # ================= batched math helpers =================

            def transpose_blocks(src_nb, tag):
                """[128, MB, D] node-major f32 -> [D, MB, 128] SBUF."""
                ht_ps = ps_s.tile([D, MB, 128], F32, tag="ht")
                for b in range(MB):
                    nc.tensor.transpose(ht_ps[:, b, :], src_nb[:, b, :],
                                        ident[:])
                ht_sb = vec.tile([D, MB, 128], F32, tag=tag)
                nc.scalar.copy(ht_sb[:], ht_ps[:])
                return ht_sb

            def sqrt_batch(dst, src_ap, tag, lo=EPS2, rdst=None):
                """dst = sqrt(max(src, lo)) via exp(0.5*ln(x));
                optionally rdst = 1/sqrt(max(src, lo)) via exp(-0.5*ln(x))."""
                clt = sc.tile(list(src_ap.shape), F32, tag=tag + "_cl")
                nc.vector.tensor_scalar_max(clt[:], src_ap, lo)
                lnt = sc.tile(list(src_ap.shape), F32, tag=tag + "_ln")
                nc.scalar.activation(lnt[:], clt[:], AF.Ln)
                nc.scalar.activation(dst, lnt[:], AF.Exp, scale=0.5)
                if rdst is not None:
                    nc.scalar.activation(rdst, lnt[:], AF.Exp, scale=-0.5)

            def div_vec(dst, num_ap, den_ap, tag):
                """dst = num/den via reciprocal+mult (ISA lacks DVE divide)."""
                rcp = sc.tile(list(den_ap.shape), F32, tag=tag + "_rc")
                nc.vector.reciprocal(rcp[:], den_ap)
                nc.vector.tensor_tensor(dst, num_ap, rcp[:], op=ALU.mult)

            def matvec_G(x_nb, ht_sb, wt_sb, g_out, fs=None, un2=None):
                """x_nb [128,MB,D] f32, ht_sb [D,MB,128] (lhsT blocks),
                wt_sb [D,D] -> g_out [128,MB,GC] f16 (mobius matvec + G).
                If fs/un2 given, |x_b|^2 = fs^2*un2 (mid_sigma shortcut)."""
                n2 = sc.tile([128, 2 * MB], F32, tag="n2")
                if fs is None:
                    sqx = vec.tile([128, MB, D], F32, tag="sqx")
                    nc.scalar.activation(sqx[:], x_nb, AF.Square)
                    nc.vector.tensor_reduce(n2[:, 0:MB], sqx[:],
                                            axis=mybir.AxisListType.X,
                                            op=ALU.add)
                else:
                    fs2m = sc.tile([128, MB], F32, tag="fs2m")
                    nc.vector.tensor_tensor(fs2m[:], fs[:], fs[:], op=ALU.mult)
                    nc.vector.tensor_tensor(n2[:, 0:MB], fs2m[:], un2[:],
                                            op=ALU.mult)

                mx_ps = ps_s.tile([128, MB, D], F32, tag="mx")
                for b in range(MB):
                    nc.tensor.matmul(mx_ps[:, b, :], ht_sb[:, b, :],
                                     wt_sb[:], start=True, stop=True)
                mxsb = vec.tile([128, MB, D], F32, tag="mxsb")
                nc.vector.tensor_copy(mxsb[:], mx_ps[:])
                sqm = vec.tile([128, MB, D], F32, tag="sqm")
                nc.scalar.activation(sqm[:], mx_ps[:], AF.Square)
                nc.vector.tensor_reduce(n2[:, MB:2 * MB], sqm[:],
                                        axis=mybir.AxisListType.X, op=ALU.add)

                nrm = sc.tile([128, 2 * MB], F32, tag="nrm")
                rnrm = sc.tile([128, 2 * MB], F32, tag="rnrm")
                sqrt_batch(nrm[:], n2[:], "nrm", rdst=rnrm[:])
                xn, mxn = nrm[:, 0:MB], nrm[:, MB:2 * MB]
                rxn, rmxn = rnrm[:, 0:MB], rnrm[:, MB:2 * MB]

                cc = sc.tile([128, MB], F32, tag="cc")
                nc.vector.tensor_scalar_min(cc[:], xn, CLIP)
                qd = sc.tile([128, MB], F32, tag="qd")
                nc.vector.tensor_scalar(qd[:], cc[:], -1.0, 1.0, op0=ALU.mult,
                                        op1=ALU.add)
                rqd = sc.tile([128, MB], F32, tag="rqd")
                nc.vector.reciprocal(rqd[:], qd[:])
                q = sc.tile([128, MB], F32, tag="q")
                nc.vector.tensor_scalar(q[:], rqd[:], 2.0, -1.0, op0=ALU.mult,
                                        op1=ALU.add)
                lnq = sc.tile([128, MB], F32, tag="lnq")
                nc.scalar.activation(lnq[:], q[:], AF.Ln)
                r = sc.tile([128, MB], F32, tag="r")
                nc.vector.tensor_tensor(r[:], mxn, rxn, op=ALU.mult)
                targ = sc.tile([128, MB], F32, tag="targ")
                nc.vector.tensor_tensor(targ[:], r[:], lnq[:], op=ALU.mult)
                Q = sc.tile([128, MB], F32, tag="Q")
                nc.scalar.activation(Q[:], targ[:], AF.Exp)
                qp1 = sc.tile([128, MB], F32, tag="qp1")
                nc.vector.tensor_scalar_add(qp1[:], Q[:], 1.0)
                rqp = sc.tile([128, MB], F32, tag="rqp")
                nc.vector.reciprocal(rqp[:], qp1[:])
                tnh = sc.tile([128, MB], F32, tag="tnh")
                nc.vector.tensor_scalar(tnh[:], rqp[:], -2.0, 1.0,
                                        op0=ALU.mult, op1=ALU.add)
                scf = sc.tile([128, MB], F32, tag="scf")
                nc.vector.tensor_tensor(scf[:], tnh[:], rmxn, op=ALU.mult)
                # s = |Y|^2 = scf^2 * mxn2 ; gamma = 2/(1-s)
                sc2 = sc.tile([128, MB], F32, tag="sc2")
                nc.vector.tensor_tensor(sc2[:], scf[:], scf[:], op=ALU.mult)
                s = sc.tile([128, MB], F32, tag="s")
                nc.vector.tensor_tensor(s[:], sc2[:], n2[:, MB:2 * MB],
                                        op=ALU.mult)
                om = sc.tile([128, MB], F32, tag="om")
                nc.vector.tensor_scalar(om[:], s[:], -1.0, 1.0, op0=ALU.mult,
                                        op1=ALU.add)
                omc = sc.tile([128, MB], F32, tag="omc")
                nc.vector.tensor_scalar_max(omc[:], om[:], EPS)
                ro = sc.tile([128, MB], F32, tag="ro")
                nc.vector.reciprocal(ro[:], omc[:])
                gs = sc.tile([128, MB], F32, tag="gs")
                nc.vector.scalar_tensor_tensor(gs[:], ro[:], 2.0, scf[:],
                                               op0=ALU.mult, op1=ALU.mult)
                for b in range(MB):
                    nc.vector.tensor_scalar_mul(g_out[:, b, 0:D],
                                                mxsb[:, b, :], gs[:, b:b + 1])
                nc.vector.tensor_scalar(g_out[:, :, D], ro[:], 2.0, -1.0,
                                        op0=ALU.mult, op1=ALU.add)
                nc.vector.memset(g_out[:, :, D + 1], 0.5)

            def mid_sigma(cblk, h_out):
                """cblk [128,MB,GC] f32 rows of (S*A)@G -> H [128,MB,D] f32.
                Returns (fs, un2) with |H_b|^2 = fs^2 * un2."""
                dv = cblk[:, :, D]          # [128, MB] strided
                ah = cblk[:, :, D + 1]
                dab = sc.tile([128, MB], F32, tag="dab")
                nc.scalar.activation(dab[:], dv, AF.Abs)
                dmx = sc.tile([128, MB], F32, tag="dmx")
                nc.vector.tensor_scalar_max(dmx[:], dab[:], 1e-10)
                sgn = sc.tile([128, MB], F32, tag="sgn")
                nc.scalar.activation(sgn[:], dv, AF.Sign)
                dsg = sc.tile([128, MB], F32, tag="dsg")
                nc.vector.tensor_tensor(dsg[:], dmx[:], sgn[:], op=ALU.mult)
                rd = sc.tile([128, MB], F32, tag="rd")
                nc.vector.reciprocal(rd[:], dsg[:])
                tm = vec.tile([128, MB, D], F32, tag="tm")
                for b in range(MB):
                    nc.vector.tensor_scalar_mul(tm[:, b, :], cblk[:, b, 0:D],
                                                rd[:, b:b + 1])
                sqt = vec.tile([128, MB, D], F32, tag="sqt")
                nc.scalar.activation(sqt[:], tm[:], AF.Square)
                t2 = sc.tile([128, MB], F32, tag="t2")
                nc.vector.tensor_reduce(t2[:], sqt[:],
                                        axis=mybir.AxisListType.X, op=ALU.add)
                omt = sc.tile([128, MB], F32, tag="omt")
                nc.vector.tensor_scalar(omt[:], t2[:], -1.0, 1.0,
                                        op0=ALU.mult, op1=ALU.add)
                sqr = sc.tile([128, MB], F32, tag="sqr")
                sqrt_batch(sqr[:], omt[:], "sqr")
                sq1 = sc.tile([128, MB], F32, tag="sq1")
                nc.vector.tensor_scalar_add(sq1[:], sqr[:], 1.0)
                rs = sc.tile([128, MB], F32, tag="rs")
                nc.vector.reciprocal(rs[:], sq1[:])
                rs2 = sc.tile([128, MB], F32, tag="rs2")
                nc.vector.tensor_tensor(rs2[:], rs[:], rs[:], op=ALU.mult)
                an2 = sc.tile([128, MB], F32, tag="an2")
                nc.vector.tensor_tensor(an2[:], rs2[:], t2[:], op=ALU.mult)
                an = sc.tile([128, MB], F32, tag="an")
                ran = sc.tile([128, MB], F32, tag="ran")
                sqrt_batch(an[:], an2[:], "an", rdst=ran[:])
                anc = sc.tile([128, MB], F32, tag="anc")
                nc.vector.tensor_scalar_min(anc[:], an[:], CLIP)
                qd2 = sc.tile([128, MB], F32, tag="qd2")
                nc.vector.tensor_scalar(qd2[:], anc[:], -1.0, 1.0,
                                        op0=ALU.mult, op1=ALU.add)
                rqd2 = sc.tile([128, MB], F32, tag="rqd2")
                nc.vector.reciprocal(rqd2[:], qd2[:])
                q2 = sc.tile([128, MB], F32, tag="q2")
                nc.vector.tensor_scalar(q2[:], rqd2[:], 2.0, -1.0,
                                        op0=ALU.mult, op1=ALU.add)
                lnq2 = sc.tile([128, MB], F32, tag="lnq2")
                nc.scalar.activation(lnq2[:], q2[:], AF.Ln)
                t2b = sc.tile([128, MB], F32, tag="t2b")
                nc.vector.scalar_tensor_tensor(t2b[:], ah, 2.0 / SCALE_A, lnq2[:],
                                               op0=ALU.mult, op1=ALU.mult)
                Q2 = sc.tile([128, MB], F32, tag="Q2")
                nc.scalar.activation(Q2[:], t2b[:], AF.Exp)
                q2p = sc.tile([128, MB], F32, tag="q2p")
                nc.vector.tensor_scalar_add(q2p[:], Q2[:], 1.0)
                rq2p = sc.tile([128, MB], F32, tag="rq2p")
                nc.vector.reciprocal(rq2p[:], q2p[:])
                th = sc.tile([128, MB], F32, tag="th")
                nc.vector.tensor_scalar(th[:], rq2p[:], -2.0, 1.0,
                                        op0=ALU.mult, op1=ALU.add)
                smo = sc.tile([128, MB], F32, tag="smo")
                nc.vector.tensor_tensor(smo[:], th[:], ran[:], op=ALU.mult)
                ms = sc.tile([128, MB], F32, tag="ms")
                nc.vector.tensor_tensor(ms[:], rs[:], smo[:], op=ALU.mult)
                ms2 = sc.tile([128, MB], F32, tag="ms2")
                nc.vector.tensor_tensor(ms2[:], ms[:], ms[:], op=ALU.mult)
                yn2 = sc.tile([128, MB], F32, tag="yn2")
                nc.vector.tensor_tensor(yn2[:], ms2[:], t2[:], op=ALU.mult)
                yn = sc.tile([128, MB], F32, tag="yn")
                ryn = sc.tile([128, MB], F32, tag="ryn")
                sqrt_batch(yn[:], yn2[:], "yn", rdst=ryn[:])
                ync = sc.tile([128, MB], F32, tag="ync")
                nc.vector.tensor_scalar_min(ync[:], yn[:], CLIP)
                qd3 = sc.tile([128, MB], F32, tag="qd3")
                nc.vector.tensor_scalar(qd3[:], ync[:], -1.0, 1.0,
                                        op0=ALU.mult, op1=ALU.add)
                rqd3 = sc.tile([128, MB], F32, tag="rqd3")
                nc.vector.reciprocal(rqd3[:], qd3[:])
                q3 = sc.tile([128, MB], F32, tag="q3")
                nc.vector.tensor_scalar(q3[:], rqd3[:], 2.0, -1.0,
                                        op0=ALU.mult, op1=ALU.add)
                lnq3 = sc.tile([128, MB], F32, tag="lnq3")
                nc.scalar.activation(lnq3[:], q3[:], AF.Ln)
                su = sc.tile([128, MB], F32, tag="su")
                nc.vector.scalar_tensor_tensor(su[:], lnq3[:], 0.5, ryn[:],
                                               op0=ALU.mult, op1=ALU.mult)
                mu = sc.tile([128, MB], F32, tag="mu")
                nc.vector.tensor_tensor(mu[:], ms[:], su[:], op=ALU.mult)
                u = vec.tile([128, MB, D], F32, tag="u")
                for b in range(MB):
                    nc.vector.tensor_scalar_mul(u[:, b, :], tm[:, b, :],
                                                mu[:, b:b + 1])
                ru = vec.tile([128, MB, D], F32, tag="ru")
                nc.scalar.activation(ru[:], u[:], AF.Relu)
                squ = vec.tile([128, MB, D], F32, tag="squ")
                nc.scalar.activation(squ[:], ru[:], AF.Square)
                un2 = sc.tile([128, MB], F32, tag="un2")
                nc.vector.tensor_reduce(un2[:], squ[:],
                                        axis=mybir.AxisListType.X, op=ALU.add)
                un = sc.tile([128, MB], F32, tag="un")
                run = sc.tile([128, MB], F32, tag="run")
                sqrt_batch(un[:], un2[:], "un", lo=4.0 * EPS2, rdst=run[:])
                E = sc.tile([128, MB], F32, tag="E")
                nc.scalar.activation(E[:], un[:], AF.Exp, scale=2.0)
                ep = sc.tile([128, MB], F32, tag="ep")
                nc.vector.tensor_scalar_add(ep[:], E[:], 1.0)
                rep = sc.tile([128, MB], F32, tag="rep")
                nc.vector.reciprocal(rep[:], ep[:])
                tt2 = sc.tile([128, MB], F32, tag="tt2")
                nc.vector.tensor_scalar(tt2[:], rep[:], -2.0, 1.0,
                                        op0=ALU.mult, op1=ALU.add)
                fs = sc.tile([128, MB], F32, tag="fs")
                nc.vector.tensor_tensor(fs[:], tt2[:], run[:], op=ALU.mult)
                for b in range(MB):
                    nc.vector.tensor_scalar_mul(h_out[:, b, :], ru[:, b, :],
                                                fs[:, b:b + 1])
                return fs, un2

            def logits_batch(ht2_sb, fs, un2, l_out):
                """H blocks (via ht2_sb lhsT) -> logits [128,MB,C] f16."""
                fs2 = sc.tile([128, MB], F32, tag="fs2")
                nc.vector.tensor_tensor(fs2[:], fs[:], fs[:], op=ALU.mult)
                y2 = sc.tile([128, MB], F32, tag="y2")
                nc.vector.tensor_tensor(y2[:], fs2[:], un2[:], op=ALU.mult)
                lg_ps = ps_s.tile([128, MB, D], F32, tag="mx")
                for b in range(MB):
                    nc.tensor.matmul(lg_ps[:, b, 0:C], ht2_sb[:, b, :],
                                     ptn_sb[:], start=True, stop=True)
                    nc.tensor.matmul(lg_ps[:, b, C:2 * C], ht2_sb[:, b, :],
                                     wlb_sb[:], start=True, stop=True)
                y2p1 = sc.tile([128, MB], F32, tag="y2p1")
                nc.vector.tensor_scalar_add(y2p1[:], y2[:], 1.0)
                alp = vec.tile([128, MB, C], F32, tag="alp")
                for b in range(MB):
                    nc.scalar.activation(alp[:, b, :], lg_ps[:, b, 0:C],
                                         AF.Identity, bias=y2p1[:, b:b + 1],
                                         scale=2.0)
                za = vec.tile([128, MB, C], F32, tag="za")
                for b in range(MB):
                    nc.vector.tensor_tensor(za[:, b, :], alp[:, b, :],
                                            cxw_sb[:], op=ALU.mult)
                    nc.vector.tensor_tensor(za[:, b, :], za[:, b, :],
                                            lg_ps[:, b, C:2 * C], op=ALU.add)
                oy = sc.tile([128, MB], F32, tag="oy")
                nc.vector.tensor_scalar(oy[:], y2[:], -1.0, 1.0, op0=ALU.mult,
                                        op1=ALU.add)
                roy = sc.tile([128, MB], F32, tag="roy")
                nc.vector.reciprocal(roy[:], oy[:])
                arg = vec.tile([128, MB, C], F32, tag="arg")
                for b in range(MB):
                    nc.vector.tensor_scalar_mul(arg[:, b, :], za[:, b, :],
                                                roy[:, b:b + 1])
                    nc.vector.tensor_tensor(arg[:, b, :], arg[:, b, :],
                                            cba_sb[:], op=ALU.mult)
                aa = vec.tile([128, MB, C], F32, tag="aa")
                nc.scalar.activation(aa[:], arg[:], AF.Abs)
                s1a = vec.tile([128, MB, C], F32, tag="s1a")
                nc.scalar.activation(s1a[:], aa[:], AF.Square)
                l1 = vec.tile([128, MB, C], F32, tag="l1")
                nc.scalar.activation(l1[:], s1a[:], AF.Ln, bias=1.0)
                s2a = vec.tile([128, MB, C], F32, tag="s2a")
                nc.scalar.activation(s2a[:], l1[:], AF.Exp, scale=0.5)
                s3a = vec.tile([128, MB, C], F32, tag="s3a")
                nc.vector.tensor_tensor(s3a[:], aa[:], s2a[:], op=ALU.add)
                dl = vec.tile([128, MB, C], F32, tag="dl")
                nc.scalar.activation(dl[:], s3a[:], AF.Ln)
                sgn2 = vec.tile([128, MB, C], F32, tag="sgn2")
                nc.scalar.activation(sgn2[:], arg[:], AF.Sign)
                dst = vec.tile([128, MB, C], F32, tag="dst")
                nc.vector.tensor_tensor(dst[:], dl[:], sgn2[:], op=ALU.mult)
                for b in range(MB):
                    nc.vector.tensor_tensor(l_out[:, b, :], dst[:, b, :],
                                            cla_sb[:], op=ALU.mult)

            def a_pass(gsb, ncols, out_ps):
                for kt in range(KT):
                    for (s0, sl) in NS:
                        nc.tensor.matmul(out_ps[:, s0:s0 + sl],
                                         gsb[:, kt, 0:ncols],
                                         at_all[:, kt, s0:s0 + sl],
                                         start=(kt == 0), stop=(kt == KT - 1))

            def c_to_blocks(cT_ps, tag):
                c_sb = vec.tile([GC, rows], F32, tag="c_sb")
                nc.scalar.copy(c_sb[:], cT_ps[:])
                tr_ps = ps_s.tile([128, MB, GC], F32, tag="tr")
                for b in range(MB):
                    nc.tensor.transpose(tr_ps[:, b, :],
                                        c_sb[:, b * 128:(b + 1) * 128],
                                        ident[0:GC, 0:GC])
                cblk = vec.tile([128, MB, GC], F32, tag=tag)
                nc.vector.tensor_copy(cblk[:], tr_ps[:])
                return cblk

            def warm_pe(n):
                wps = ps_s.tile([128, MB, D], F32, tag="mx")
                wv = wps[:].rearrange("p a b -> p (a b)")
                for _ in range(n):
                    nc.tensor.matmul(wv, warm_l[:], warm_r[:],
                                     start=True, stop=True)

            # ================= phase 0: X -> G1 -> allgather =================
            g1_t = vec.tile([128, MB, GC], F16, tag="g_t")
            matvec_G(x_sb[:], xt_sb[:].rearrange("d (b p) -> d b p", p=128),
                     w1t_sb, g1_t)
            nc.sync.dma_start(out=g1_loc[:, :, :], in_=g1_t[:])
            nc.gpsimd.collective_compute(
                "AllGather", ALU.bypass, replica_groups=rg,
                ins=[g1_loc[:, :, :].opt()], outs=[g1_full[:, :, :, :].opt()])
            nc.sync.dma_start(
                out=g1sb[:].rearrange("p (i b) c -> p i b c", i=ncores),
                in_=g1_full[:, :, :, :].rearrange("i p b c -> p i b c"))
            warm_pe(150)

            # ================= pass 1 =================
            c1_ps = ps_c.tile([GC, rows], F32, tag="acc")
            a_pass(g1sb, GC, c1_ps)

            # ================= boundary 1: C1 -> H1 -> G2 =================
            cblk1 = c_to_blocks(c1_ps, "cblk")
            warm_pe(44)
            h1 = vec.tile([128, MB, D], F32, tag="h")
            fs1, un21 = mid_sigma(cblk1, h1)
            ht1 = transpose_blocks(h1, "ht1")
            g2_t = vec.tile([128, MB, GC], F16, tag="g_t")
            matvec_G(h1[:], ht1[:], w2t_sb, g2_t, fs=fs1, un2=un21)
            nc.sync.dma_start(out=g2_loc[:, :, :], in_=g2_t[:])
            nc.gpsimd.collective_compute(
                "AllGather", ALU.bypass, replica_groups=rg,
                ins=[g2_loc[:, :, :].opt()], outs=[g2_full[:, :, :, :].opt()])
            nc.sync.dma_start(
                out=g2sb[:].rearrange("p (i b) c -> p i b c", i=ncores),
                in_=g2_full[:, :, :, :].rearrange("i p b c -> p i b c"))
            warm_pe(60)

            # ================= pass 2 =================
            c2_ps = ps_c.tile([GC, rows], F32, tag="acc")
            a_pass(g2sb, GC, c2_ps)

            # ================= boundary 2: C2 -> H2 -> logits =================
            cblk2 = c_to_blocks(c2_ps, "cblk")
            warm_pe(44)
            h2 = vec.tile([128, MB, D], F32, tag="h")
            fs, un2 = mid_sigma(cblk2, h2)
            ht2 = transpose_blocks(h2, "ht2")
            l_t = vec.tile([128, MB, C], F16, tag="l_t")
            logits_batch(ht2, fs, un2, l_t)
            nc.sync.dma_start(out=l_loc[:, :, :], in_=l_t[:])
            nc.gpsimd.collective_compute(
                "AllGather", ALU.bypass, replica_groups=rg,
                ins=[l_loc[:, :, :].opt()], outs=[l_full[:, :, :, :].opt()])
            nc.sync.dma_start(
                out=lsb[:].rearrange("p (i b) c -> p i b c", i=ncores),
                in_=l_full[:, :, :, :].rearrange("i p b c -> p i b c"))
            warm_pe(40)

            # ================= pass 3: out^T = L^T A^T =================
            o_ps = ps_c.tile([C, rows], F32, tag="o")
            for kt in range(KT):
                for (s0, sl) in NS:
                    nc.tensor.matmul(o_ps[:, s0:s0 + sl], lsb[:, kt, :],
                                     at_all[:, kt, s0:s0 + sl],
                                     start=(kt == 0), stop=(kt == KT - 1))
            o_sb = singles.tile([C, rows], F32, tag="o_sb")
            nc.scalar.activation(o_sb[:], o_ps[:], AF.Copy,
                                 scale=1.0 / SCALE_A)
            nc.sync.dma_start(out=outT[:, :], in_=o_sb[:])

    _split_multiwaits(nc)
    return nc


def _host_inputs(X, A_hat, W1, W2, W_logits, p_ks, N=N_FULL, ncores=NCORES):
    rows = N // ncores
    MB = rows // 128
    KT = N // 128
    f = np.float32
    from ml_dtypes import float8_e4m3fn as f8

    X = np.ascontiguousarray(X, f)
    A_hat = np.ascontiguousarray(A_hat, f)
    AT = np.ascontiguousarray(A_hat.T)
    W1 = np.asarray(W1, f)
    W2 = np.asarray(W2, f)
    WL = np.asarray(W_logits, f)
    PK = np.asarray(p_ks, f)

    x2 = np.sum(PK * PK, axis=-1)                       # |p_k|^2
    a_norm = np.maximum(np.sqrt(np.sum(WL * WL, 0)), 1e-10)
    beta = 1.0 - x2
    xW = np.einsum('kd,dk->k', -PK, WL)
    lam = 2.0 / np.maximum(1.0 - x2, EPS)

    shared = {
        "W1t": np.ascontiguousarray(W1.T, f),
        "W2t": np.ascontiguousarray(W2.T, f),
        "WLb": np.ascontiguousarray(WL * beta[None, :], f),
        "PTn": np.ascontiguousarray(-PK.T, f),
        "cXW": xW.reshape(1, C).astype(f),
        "cBA": (2.0 / (beta * a_norm)).reshape(1, C).astype(f),
        "cLA": (lam * a_norm).reshape(1, C).astype(f),
    }
    in_maps = []
    for i in range(ncores):
        bi = i * rows
        at = np.empty((128, KT, rows), f8)
        for t in range(KT):
            at[:, t, :] = AT[t * 128:(t + 1) * 128, bi:bi + rows] * SCALE_A
        Xl = X[bi:bi + rows]
        m = dict(shared)
        m["At"] = at
        m["Xp"] = np.ascontiguousarray(
            Xl.reshape(MB, 128, D).transpose(1, 0, 2))
        m["Xt"] = np.ascontiguousarray(Xl.T)
        in_maps.append(m)
    return in_maps


_PROGRAM_CACHE = {}


def _get_program(N=N_FULL, ncores=NCORES):
    key = (N, ncores)
    if key not in _PROGRAM_CACHE:
        _PROGRAM_CACHE[key] = build_program(N, ncores)
    return _PROGRAM_CACHE[key]


def run(inputs, trace=False, N=N_FULL, ncores=NCORES):
    nc = _get_program(N, ncores)
    in_maps = _host_inputs(N=N, ncores=ncores, **inputs)
    res = run_bass_kernel_spmd(nc, in_maps, core_ids=list(range(ncores)),
                               trace=trace)
    out = np.concatenate([np.ascontiguousarray(res.results[i]["outT"]).T
                          for i in range(ncores)], axis=0)
    return out.astype(np.float32), res


def kernel(X, A_hat, W1, W2, W_logits, p_ks):
    out, _ = run(dict(X=X, A_hat=A_hat, W1=W1, W2=W2,
                      W_logits=W_logits, p_ks=p_ks))
    return out


# revision 17
# speedup vs baseline: 1.0500x; 1.0500x over previous
"""KappaGCN (Poincare ball, K=-1) on 8 Trainium2 NeuronCores.

Sharding: rows of A / X over cores (1024 nodes/core). Layout choices that
carry the speed:
- The host ships each core its A slice PRE-TRANSPOSED (A^T[:, rows_i]),
  scaled by 8192 and cast to fp8-e4m3 (TRN e4 max-normal 240 >> 2.4 max),
  laid out exactly as the resident SBUF tile [128, 64 k-tiles, 1024] - zero
  on-device transposes of A, one 8 MB HBM read serving all three A@(.)
  passes (fp16/fp8 matmuls at 1 cycle/row, fp32 PSUM accumulation).
- Every scalar-engine activation comes from the single natural_log_exp
  table set: sqrt(x)=exp(0.5 ln x), rsqrt free via exp(-0.5 ln x),
  tanh(t)=1-2/(e^2t+1) - exactly one ACT_TABLE_LOAD in the whole kernel.
- Per-node math is batched across node blocks ([128, blocks] tiles), with
  norms via one Square + one segmented vector reduce.
- Boundaries are split into half-batches pipelined against the pass
  512-column PSUM chunks; G2/logits halves AllGather as soon as each
  half is ready so gathers overlap the other half's compute. G rides
  the wire as fp8 pairs bitcast onto f16 collective buffers (halved
  gather bytes); logits stay f16 (their fp8 path miscomputes).
- A gpsimd marker copy holds the 8 MB A^T stream until the small phase-0
  inputs land, so the DMA firehose cannot starve the critical path.

Per core: G1 = f(X) -> AllGather -> C1^T = G1^T A_i^T -> H1 -> G2
          -> AllGather x2 (halves) -> C2^T -> H2 -> logits L
          -> AllGather x2 -> out^T = L^T A_i^T.
"""
import os
import sys
import numpy as np

os.environ.setdefault("NEURON_RT_RESET_CORES", "1")
os.environ.setdefault("MYCRO_LOCAL_CACHE", "1")

for _p in ("/opt/trn_rl_repo",):
    if _p not in sys.path:
        sys.path.insert(0, _p)

import concourse.bass as bass
import concourse.mybir as mybir
import concourse.tile as tile
from concourse.masks import make_identity
from concourse.bass_utils import run_bass_kernel_spmd

F32 = mybir.dt.float32
F16 = mybir.dt.float16
F8 = mybir.dt.float8e4
AF = mybir.ActivationFunctionType
ALU = mybir.AluOpType

N_FULL = 8192
D = 32
C = 16
NCORES = 8
GC = D + 2          # G columns: [gamma*Y (32) | gamma-1 | 0.5]
CLIP = 1.0 - 1e-7
EPS = 1e-15
EPS2 = 1e-30        # clamp on squared norms == EPS clamp on norms
SCALE_A = 8192.0    # A premultiplied by this on host (fp16 normal range)


def _split_multiwaits(nc, limit=1):
    """This toolchain's walrus rejects instructions with more than `limit`
    sync waits; peel excess waits onto standalone EventSemaphore carriers
    placed just before, on the same engine queue (order-preserving)."""
    n_new = 0
    for bb in nc.main_func.blocks:
        out = []
        changed = False
        for ins in bb.instructions:
            si = getattr(ins, "sync_info", None)
            waits = list(si.on_wait) if si is not None and si.on_wait else []
            if len(waits) > limit:
                changed = True
                excess, keep = waits[:-limit], waits[-limit:]
                for i in range(0, len(excess), limit):
                    n_new += 1
                    out.append(mybir.InstEventSemaphore(
                        name=f"mwsplit_{n_new}_{ins.name}",
                        engine=ins.engine,
                        ins=[], outs=[],
                        sync_info=mybir.SyncInfo(
                            on_wait=excess[i:i + limit], on_update=[]),
                    ))
                try:
                    si.on_wait = keep
                except Exception:
                    ins.sync_info = mybir.SyncInfo(
                        on_wait=keep, on_update=list(si.on_update))
            out.append(ins)
        if changed:
            try:
                bb.instructions[:] = out
            except Exception:
                bb.set_instructions(out)
    return n_new


def build_program(N=N_FULL, ncores=NCORES):
    rows = N // ncores          # nodes per core
    MB = rows // 128            # node blocks per core
    KT = N // 128               # contraction tiles
    NS = [(s, min(512, rows - s)) for s in range(0, rows, 512)]
    CH = 8                      # A^T DMA chunks
    KC = KT // CH

    nc = bass.Bass(num_devices=ncores)

    At = nc.dram_tensor("At", [128, KT, rows], F8, kind="ExternalInput")
    Xp = nc.dram_tensor("Xp", [128, MB, D], F32, kind="ExternalInput")
    Xt = nc.dram_tensor("Xt", [D, rows], F32, kind="ExternalInput")
    W1t = nc.dram_tensor("W1t", [D, D], F32, kind="ExternalInput")
    W2t = nc.dram_tensor("W2t", [D, D], F32, kind="ExternalInput")
    WLb = nc.dram_tensor("WLb", [D, C], F32, kind="ExternalInput")
    PTn = nc.dram_tensor("PTn", [D, C], F32, kind="ExternalInput")
    cXW = nc.dram_tensor("cXW", [1, C], F32, kind="ExternalInput")
    cBA = nc.dram_tensor("cBA", [1, C], F32, kind="ExternalInput")
    cLA = nc.dram_tensor("cLA", [1, C], F32, kind="ExternalInput")
    outT = nc.dram_tensor("outT", [C, rows], F32, kind="ExternalOutput")

    g1_loc = nc.dram_tensor("g1_loc", [128, MB, GC // 2], F16)
    g1_full = nc.dram_tensor("g1_full", [ncores, 128, MB, GC // 2], F16,
                             addr_space="Shared")
    g2_loc = {}
    g2_full = {}
    l_loc = {}
    l_full = {}
    for h in (0, 1):
        g2_loc[h] = nc.dram_tensor(f"g2_loc{h}", [128, HB, GC // 2], F16)
        g2_full[h] = nc.dram_tensor(f"g2_full{h}", [ncores, 128, HB, GC // 2],
                                    F16, addr_space="Shared")
        l_loc[h] = nc.dram_tensor(f"l_loc{h}", [128, HB, C], F16)
        l_full[h] = nc.dram_tensor(f"l_full{h}", [ncores, 128, HB, C], F16,
                                   addr_space="Shared")
    rg = [list(range(ncores))]

    with tile.TileContext(nc, num_cores=ncores) as tc:
        import contextlib
        with contextlib.ExitStack() as ctx:
            singles = ctx.enter_context(tc.tile_pool(name="singles", bufs=1))
            sc = ctx.enter_context(tc.tile_pool(name="sc", bufs=1))
            vec = ctx.enter_context(tc.tile_pool(name="vec", bufs=1))
            ps_c = ctx.enter_context(tc.tile_pool(name="ps_c", bufs=1,
                                                  space="PSUM"))
            ps_s = ctx.enter_context(tc.tile_pool(name="ps_s", bufs=1,
                                                  space="PSUM"))

            # ---- small loads (sync queue, issued first) ----
            ident = singles.tile([128, 128], F32)
            make_identity(nc, ident[:])
            w1t_sb = singles.tile([D, D], F32, tag="w1t")
            nc.sync.dma_start(out=w1t_sb[:], in_=W1t[:, :])
            w2t_sb = singles.tile([D, D], F32, tag="w2t")
            nc.sync.dma_start(out=w2t_sb[:], in_=W2t[:, :])
            wlb_sb = singles.tile([D, C], F32, tag="wlb")
            nc.sync.dma_start(out=wlb_sb[:], in_=WLb[:, :])
            ptn_sb = singles.tile([D, C], F32, tag="ptn")
            nc.sync.dma_start(out=ptn_sb[:], in_=PTn[:, :])

            def bcast(dram):
                t = singles.tile([128, C], F32, tag=dram.name)
                nc.sync.dma_start(out=t[:],
                                  in_=bass.AP(dram, 0, [[0, 128], [1, C]]))
                return t
            cxw_sb = bcast(cXW)
            cba_sb = bcast(cBA)
            cla_sb = bcast(cLA)

            x_sb = singles.tile([128, MB, D], F32, tag="x_sb")
            nc.sync.dma_start(out=x_sb[:], in_=Xp[:, :, :])
            xt_sb = singles.tile([D, rows], F32, tag="xt_sb")
            nc.sync.dma_start(out=xt_sb[:], in_=Xt[:, :])

            # ---- A^T streaming into resident SBUF (gpsimd queue) ----
            # The gpsimd marker copy blocks the gpsimd engine until the small
            # loads land, so the 16MB A^T stream can't starve them of DMA
            # bandwidth (phase0 needs x/xt/weights immediately).
            marker = singles.tile([1, 4], F32, tag="marker")
            nc.gpsimd.tensor_copy(marker[:], xt_sb[0:1, 0:4])
            nc.gpsimd.tensor_copy(marker[:], x_sb[0:1, 0, 0:4])
            nc.gpsimd.tensor_copy(marker[:], w1t_sb[0:1, 0:4])
            at_all = singles.tile([128, KT, rows], F8, tag="at_all")
            for cch in range(CH):
                nc.gpsimd.dma_start(
                    out=at_all[:, cch * KC:(cch + 1) * KC, :],
                    in_=At[:, cch * KC:(cch + 1) * KC, :])

            # Junk operands for PE warm-keeper matmuls (hold the HAM clock
            # gate at 8/8 through otherwise PE-idle stretches). M=128 so the
            # activity monitor sees a busy array.
            warm_l = singles.tile([128, 128], F16, tag="warm_l")
            nc.vector.memset(warm_l[:], 0.0)
            warm_r = singles.tile([128, 256], F16, tag="warm_r")
            nc.vector.memset(warm_r[:], 0.0)

            g1sb = singles.tile([128, KT, GC], F16, tag="g1sb")
            g2sb = singles.tile([128, KT, GC], F16, tag="g2sb")
            lsb = singles.tile([128, KT, C], F16, tag="lsb")

            # ================= batched math helpers =================

            def transpose_blocks(src_nb, tag):
                """[128, MB, D] node-major f32 -> [D, MB, 128] SBUF."""
                ht_ps = ps_s.tile([D, MB, 128], F32, tag="ht")
                for b in range(MB):
                    nc.tensor.transpose(ht_ps[:, b, :], src_nb[:, b, :],
                                        ident[:])
                ht_sb = vec.tile([D, MB, 128], F32, tag=tag)
                nc.scalar.copy(ht_sb[:], ht_ps[:])
                return ht_sb

            def sqrt_batch(dst, src_ap, tag, lo=EPS2, rdst=None):
                """dst = sqrt(max(src, lo)) via exp(0.5*ln(x));
                optionally rdst = 1/sqrt(max(src, lo)) via exp(-0.5*ln(x))."""
                clt = sc.tile(list(src_ap.shape), F32, tag=tag + "_cl")
                nc.vector.tensor_scalar_max(clt[:], src_ap, lo)
                lnt = sc.tile(list(src_ap.shape), F32, tag=tag + "_ln")
                nc.scalar.activation(lnt[:], clt[:], AF.Ln)
                nc.scalar.activation(dst, lnt[:], AF.Exp, scale=0.5)
                if rdst is not None:
                    nc.scalar.activation(rdst, lnt[:], AF.Exp, scale=-0.5)

            def div_vec(dst, num_ap, den_ap, tag):
                """dst = num/den via reciprocal+mult (ISA lacks DVE divide)."""
                rcp = sc.tile(list(den_ap.shape), F32, tag=tag + "_rc")
                nc.vector.reciprocal(rcp[:], den_ap)
                nc.vector.tensor_tensor(dst, num_ap, rcp[:], op=ALU.mult)

            def matvec_G(x_nb, ht_sb, wt_sb, g_out, fs=None, un2=None):
                """x_nb [128,MB,D] f32, ht_sb [D,MB,128] (lhsT blocks),
                wt_sb [D,D] -> g_out [128,MB,GC] f16 (mobius matvec + G).
                If fs/un2 given, |x_b|^2 = fs^2*un2 (mid_sigma shortcut)."""
                n2 = sc.tile([128, 2 * MB], F32, tag="n2")
                if fs is None:
                    sqx = vec.tile([128, MB, D], F32, tag="sqx")
                    nc.scalar.activation(sqx[:], x_nb, AF.Square)
                    nc.vector.tensor_reduce(n2[:, 0:MB], sqx[:],
                                            axis=mybir.AxisListType.X,
                                            op=ALU.add)
                else:
                    fs2m = sc.tile([128, MB], F32, tag="fs2m")
                    nc.vector.tensor_tensor(fs2m[:], fs[:], fs[:], op=ALU.mult)
                    nc.vector.tensor_tensor(n2[:, 0:MB], fs2m[:], un2[:],
                                            op=ALU.mult)

                mx_ps = ps_s.tile([128, MB, D], F32, tag="mx")
                for b in range(MB):
                    nc.tensor.matmul(mx_ps[:, b, :], ht_sb[:, b, :],
                                     wt_sb[:], start=True, stop=True)
                mxsb = vec.tile([128, MB, D], F32, tag="mxsb")
                nc.vector.tensor_copy(mxsb[:], mx_ps[:])
                sqm = vec.tile([128, MB, D], F32, tag="sqm")
                nc.scalar.activation(sqm[:], mx_ps[:], AF.Square)
                nc.vector.tensor_reduce(n2[:, MB:2 * MB], sqm[:],
                                        axis=mybir.AxisListType.X, op=ALU.add)

                nrm = sc.tile([128, 2 * MB], F32, tag="nrm")
                rnrm = sc.tile([128, 2 * MB], F32, tag="rnrm")
                sqrt_batch(nrm[:], n2[:], "nrm", rdst=rnrm[:])
                xn, mxn = nrm[:, 0:MB], nrm[:, MB:2 * MB]
                rxn, rmxn = rnrm[:, 0:MB], rnrm[:, MB:2 * MB]

                cc = sc.tile([128, MB], F32, tag="cc")
                nc.vector.tensor_scalar_min(cc[:], xn, CLIP)
                qd = sc.tile([128, MB], F32, tag="qd")
                nc.vector.tensor_scalar(qd[:], cc[:], -1.0, 1.0, op0=ALU.mult,
                                        op1=ALU.add)
                rqd = sc.tile([128, MB], F32, tag="rqd")
                nc.vector.reciprocal(rqd[:], qd[:])
                q = sc.tile([128, MB], F32, tag="q")
                nc.vector.tensor_scalar(q[:], rqd[:], 2.0, -1.0, op0=ALU.mult,
                                        op1=ALU.add)
                lnq = sc.tile([128, MB], F32, tag="lnq")
                nc.scalar.activation(lnq[:], q[:], AF.Ln)
                r = sc.tile([128, MB], F32, tag="r")
                nc.vector.tensor_tensor(r[:], mxn, rxn, op=ALU.mult)
                targ = sc.tile([128, MB], F32, tag="targ")
                nc.vector.tensor_tensor(targ[:], r[:], lnq[:], op=ALU.mult)
                Q = sc.tile([128, MB], F32, tag="Q")
                nc.scalar.activation(Q[:], targ[:], AF.Exp)
                qp1 = sc.tile([128, MB], F32, tag="qp1")
                nc.vector.tensor_scalar_add(qp1[:], Q[:], 1.0)
                rqp = sc.tile([128, MB], F32, tag="rqp")
                nc.vector.reciprocal(rqp[:], qp1[:])
                tnh = sc.tile([128, MB], F32, tag="tnh")
                nc.vector.tensor_scalar(tnh[:], rqp[:], -2.0, 1.0,
                                        op0=ALU.mult, op1=ALU.add)
                scf = sc.tile([128, MB], F32, tag="scf")
                nc.vector.tensor_tensor(scf[:], tnh[:], rmxn, op=ALU.mult)
                # s = |Y|^2 = scf^2 * mxn2 ; gamma = 2/(1-s)
                sc2 = sc.tile([128, MB], F32, tag="sc2")
                nc.vector.tensor_tensor(sc2[:], scf[:], scf[:], op=ALU.mult)
                s = sc.tile([128, MB], F32, tag="s")
                nc.vector.tensor_tensor(s[:], sc2[:], n2[:, MB:2 * MB],
                                        op=ALU.mult)
                om = sc.tile([128, MB], F32, tag="om")
                nc.vector.tensor_scalar(om[:], s[:], -1.0, 1.0, op0=ALU.mult,
                                        op1=ALU.add)
                omc = sc.tile([128, MB], F32, tag="omc")
                nc.vector.tensor_scalar_max(omc[:], om[:], EPS)
                ro = sc.tile([128, MB], F32, tag="ro")
                nc.vector.reciprocal(ro[:], omc[:])
                gs = sc.tile([128, MB], F32, tag="gs")
                nc.vector.scalar_tensor_tensor(gs[:], ro[:], 2.0, scf[:],
                                               op0=ALU.mult, op1=ALU.mult)
                for b in range(MB):
                    nc.vector.tensor_scalar_mul(g_out[:, b, 0:D],
                                                mxsb[:, b, :], gs[:, b:b + 1])
                nc.vector.tensor_scalar(g_out[:, :, D], ro[:], 2.0, -1.0,
                                        op0=ALU.mult, op1=ALU.add)
                nc.vector.memset(g_out[:, :, D + 1], 0.5)

            def mid_sigma(cblk, h_out):
                """cblk [128,MB,GC] f32 rows of (S*A)@G -> H [128,MB,D] f32.
                Returns (fs, un2) with |H_b|^2 = fs^2 * un2."""
                dv = cblk[:, :, D]          # [128, MB] strided
                ah = cblk[:, :, D + 1]
                dab = sc.tile([128, MB], F32, tag="dab")
                nc.scalar.activation(dab[:], dv, AF.Abs)
                dmx = sc.tile([128, MB], F32, tag="dmx")
                nc.vector.tensor_scalar_max(dmx[:], dab[:], 1e-10)
                sgn = sc.tile([128, MB], F32, tag="sgn")
                nc.scalar.activation(sgn[:], dv, AF.Sign)
                dsg = sc.tile([128, MB], F32, tag="dsg")
                nc.vector.tensor_tensor(dsg[:], dmx[:], sgn[:], op=ALU.mult)
                rd = sc.tile([128, MB], F32, tag="rd")
                nc.vector.reciprocal(rd[:], dsg[:])
                tm = vec.tile([128, MB, D], F32, tag="tm")
                for b in range(MB):
                    nc.vector.tensor_scalar_mul(tm[:, b, :], cblk[:, b, 0:D],
                                                rd[:, b:b + 1])
                sqt = vec.tile([128, MB, D], F32, tag="sqt")
                nc.scalar.activation(sqt[:], tm[:], AF.Square)
                t2 = sc.tile([128, MB], F32, tag="t2")
                nc.vector.tensor_reduce(t2[:], sqt[:],
                                        axis=mybir.AxisListType.X, op=ALU.add)
                omt = sc.tile([128, MB], F32, tag="omt")
                nc.vector.tensor_scalar(omt[:], t2[:], -1.0, 1.0,
                                        op0=ALU.mult, op1=ALU.add)
                sqr = sc.tile([128, MB], F32, tag="sqr")
                sqrt_batch(sqr[:], omt[:], "sqr")
                sq1 = sc.tile([128, MB], F32, tag="sq1")
                nc.vector.tensor_scalar_add(sq1[:], sqr[:], 1.0)
                rs = sc.tile([128, MB], F32, tag="rs")
                nc.vector.reciprocal(rs[:], sq1[:])
                rs2 = sc.tile([128, MB], F32, tag="rs2")
                nc.vector.tensor_tensor(rs2[:], rs[:], rs[:], op=ALU.mult)
                an2 = sc.tile([128, MB], F32, tag="an2")
                nc.vector.tensor_tensor(an2[:], rs2[:], t2[:], op=ALU.mult)
                an = sc.tile([128, MB], F32, tag="an")
                ran = sc.tile([128, MB], F32, tag="ran")
                sqrt_batch(an[:], an2[:], "an", rdst=ran[:])
                anc = sc.tile([128, MB], F32, tag="anc")
                nc.vector.tensor_scalar_min(anc[:], an[:], CLIP)
                qd2 = sc.tile([128, MB], F32, tag="qd2")
                nc.vector.tensor_scalar(qd2[:], anc[:], -1.0, 1.0,
                                        op0=ALU.mult, op1=ALU.add)
                rqd2 = sc.tile([128, MB], F32, tag="rqd2")
                nc.vector.reciprocal(rqd2[:], qd2[:])
                q2 = sc.tile([128, MB], F32, tag="q2")
                nc.vector.tensor_scalar(q2[:], rqd2[:], 2.0, -1.0,
                                        op0=ALU.mult, op1=ALU.add)
                lnq2 = sc.tile([128, MB], F32, tag="lnq2")
                nc.scalar.activation(lnq2[:], q2[:], AF.Ln)
                t2b = sc.tile([128, MB], F32, tag="t2b")
                nc.vector.scalar_tensor_tensor(t2b[:], ah, 2.0 / SCALE_A, lnq2[:],
                                               op0=ALU.mult, op1=ALU.mult)
                Q2 = sc.tile([128, MB], F32, tag="Q2")
                nc.scalar.activation(Q2[:], t2b[:], AF.Exp)
                q2p = sc.tile([128, MB], F32, tag="q2p")
                nc.vector.tensor_scalar_add(q2p[:], Q2[:], 1.0)
                rq2p = sc.tile([128, MB], F32, tag="rq2p")
                nc.vector.reciprocal(rq2p[:], q2p[:])
                th = sc.tile([128, MB], F32, tag="th")
                nc.vector.tensor_scalar(th[:], rq2p[:], -2.0, 1.0,
                                        op0=ALU.mult, op1=ALU.add)
                smo = sc.tile([128, MB], F32, tag="smo")
                nc.vector.tensor_tensor(smo[:], th[:], ran[:], op=ALU.mult)
                ms = sc.tile([128, MB], F32, tag="ms")
                nc.vector.tensor_tensor(ms[:], rs[:], smo[:], op=ALU.mult)
                ms2 = sc.tile([128, MB], F32, tag="ms2")
                nc.vector.tensor_tensor(ms2[:], ms[:], ms[:], op=ALU.mult)
                yn2 = sc.tile([128, MB], F32, tag="yn2")
                nc.vector.tensor_tensor(yn2[:], ms2[:], t2[:], op=ALU.mult)
                yn = sc.tile([128, MB], F32, tag="yn")
                ryn = sc.tile([128, MB], F32, tag="ryn")
                sqrt_batch(yn[:], yn2[:], "yn", rdst=ryn[:])
                ync = sc.tile([128, MB], F32, tag="ync")
                nc.vector.tensor_scalar_min(ync[:], yn[:], CLIP)
                qd3 = sc.tile([128, MB], F32, tag="qd3")
                nc.vector.tensor_scalar(qd3[:], ync[:], -1.0, 1.0,
                                        op0=ALU.mult, op1=ALU.add)
                rqd3 = sc.tile([128, MB], F32, tag="rqd3")
                nc.vector.reciprocal(rqd3[:], qd3[:])
                q3 = sc.tile([128, MB], F32, tag="q3")
                nc.vector.tensor_scalar(q3[:], rqd3[:], 2.0, -1.0,
                                        op0=ALU.mult, op1=ALU.add)
                lnq3 = sc.tile([128, MB], F32, tag="lnq3")
                nc.scalar.activation(lnq3[:], q3[:], AF.Ln)
                su = sc.tile([128, MB], F32, tag="su")
                nc.vector.scalar_tensor_tensor(su[:], lnq3[:], 0.5, ryn[:],
                                               op0=ALU.mult, op1=ALU.mult)
                mu = sc.tile([128, MB], F32, tag="mu")
                nc.vector.tensor_tensor(mu[:], ms[:], su[:], op=ALU.mult)
                u = vec.tile([128, MB, D], F32, tag="u")
                for b in range(MB):
                    nc.vector.tensor_scalar_mul(u[:, b, :], tm[:, b, :],
                                                mu[:, b:b + 1])
                ru = vec.tile([128, MB, D], F32, tag="ru")
                nc.scalar.activation(ru[:], u[:], AF.Relu)
                squ = vec.tile([128, MB, D], F32, tag="squ")
                nc.scalar.activation(squ[:], ru[:], AF.Square)
                un2 = sc.tile([128, MB], F32, tag="un2")
                nc.vector.tensor_reduce(un2[:], squ[:],
                                        axis=mybir.AxisListType.X, op=ALU.add)
                un = sc.tile([128, MB], F32, tag="un")
                run = sc.tile([128, MB], F32, tag="run")
                sqrt_batch(un[:], un2[:], "un", lo=4.0 * EPS2, rdst=run[:])
                E = sc.tile([128, MB], F32, tag="E")
                nc.scalar.activation(E[:], un[:], AF.Exp, scale=2.0)
                ep = sc.tile([128, MB], F32, tag="ep")
                nc.vector.tensor_scalar_add(ep[:], E[:], 1.0)
                rep = sc.tile([128, MB], F32, tag="rep")
                nc.vector.reciprocal(rep[:], ep[:])
                tt2 = sc.tile([128, MB], F32, tag="tt2")
                nc.vector.tensor_scalar(tt2[:], rep[:], -2.0, 1.0,
                                        op0=ALU.mult, op1=ALU.add)
                fs = sc.tile([128, MB], F32, tag="fs")
                nc.vector.tensor_tensor(fs[:], tt2[:], run[:], op=ALU.mult)
                for b in range(MB):
                    nc.vector.tensor_scalar_mul(h_out[:, b, :], ru[:, b, :],
                                                fs[:, b:b + 1])
                return fs, un2

            def logits_batch(ht2_sb, fs, un2, l_out):
                """H blocks (via ht2_sb lhsT) -> logits [128,MB,C] f16."""
                fs2 = sc.tile([128, MB], F32, tag="fs2")
                nc.vector.tensor_tensor(fs2[:], fs[:], fs[:], op=ALU.mult)
                y2 = sc.tile([128, MB], F32, tag="y2")
                nc.vector.tensor_tensor(y2[:], fs2[:], un2[:], op=ALU.mult)
                lg_ps = ps_s.tile([128, MB, D], F32, tag="mx")
                for b in range(MB):
                    nc.tensor.matmul(lg_ps[:, b, 0:C], ht2_sb[:, b, :],
                                     ptn_sb[:], start=True, stop=True)
                    nc.tensor.matmul(lg_ps[:, b, C:2 * C], ht2_sb[:, b, :],
                                     wlb_sb[:], start=True, stop=True)
                y2p1 = sc.tile([128, MB], F32, tag="y2p1")
                nc.vector.tensor_scalar_add(y2p1[:], y2[:], 1.0)
                alp = vec.tile([128, MB, C], F32, tag="alp")
                for b in range(MB):
                    nc.scalar.activation(alp[:, b, :], lg_ps[:, b, 0:C],
                                         AF.Identity, bias=y2p1[:, b:b + 1],
                                         scale=2.0)
                za = vec.tile([128, MB, C], F32, tag="za")
                for b in range(MB):
                    nc.vector.tensor_tensor(za[:, b, :], alp[:, b, :],
                                            cxw_sb[:], op=ALU.mult)
                    nc.vector.tensor_tensor(za[:, b, :], za[:, b, :],
                                            lg_ps[:, b, C:2 * C], op=ALU.add)
                oy = sc.tile([128, MB], F32, tag="oy")
                nc.vector.tensor_scalar(oy[:], y2[:], -1.0, 1.0, op0=ALU.mult,
                                        op1=ALU.add)
                roy = sc.tile([128, MB], F32, tag="roy")
                nc.vector.reciprocal(roy[:], oy[:])
                arg = vec.tile([128, MB, C], F32, tag="arg")
                for b in range(MB):
                    nc.vector.tensor_scalar_mul(arg[:, b, :], za[:, b, :],
                                                roy[:, b:b + 1])
                    nc.vector.tensor_tensor(arg[:, b, :], arg[:, b, :],
                                            cba_sb[:], op=ALU.mult)
                aa = vec.tile([128, MB, C], F32, tag="aa")
                nc.scalar.activation(aa[:], arg[:], AF.Abs)
                s1a = vec.tile([128, MB, C], F32, tag="s1a")
                nc.scalar.activation(s1a[:], aa[:], AF.Square)
                l1 = vec.tile([128, MB, C], F32, tag="l1")
                nc.scalar.activation(l1[:], s1a[:], AF.Ln, bias=1.0)
                s2a = vec.tile([128, MB, C], F32, tag="s2a")
                nc.scalar.activation(s2a[:], l1[:], AF.Exp, scale=0.5)
                s3a = vec.tile([128, MB, C], F32, tag="s3a")
                nc.vector.tensor_tensor(s3a[:], aa[:], s2a[:], op=ALU.add)
                dl = vec.tile([128, MB, C], F32, tag="dl")
                nc.scalar.activation(dl[:], s3a[:], AF.Ln)
                sgn2 = vec.tile([128, MB, C], F32, tag="sgn2")
                nc.scalar.activation(sgn2[:], arg[:], AF.Sign)
                dst = vec.tile([128, MB, C], F32, tag="dst")
                nc.vector.tensor_tensor(dst[:], dl[:], sgn2[:], op=ALU.mult)
                for b in range(MB):
                    nc.vector.tensor_tensor(l_out[:, b, :], dst[:, b, :],
                                            cla_sb[:], op=ALU.mult)

            def a_pass(gsb, ncols, out_ps):
                for kt in range(KT):
                    for (s0, sl) in NS:
                        nc.tensor.matmul(out_ps[:, s0:s0 + sl],
                                         gsb[:, kt, 0:ncols],
                                         at_all[:, kt, s0:s0 + sl],
                                         start=(kt == 0), stop=(kt == KT - 1))

            def c_to_blocks(cT_ps, tag):
                c_sb = vec.tile([GC, rows], F32, tag="c_sb")
                nc.scalar.copy(c_sb[:], cT_ps[:])
                tr_ps = ps_s.tile([128, MB, GC], F32, tag="tr")
                for b in range(MB):
                    nc.tensor.transpose(tr_ps[:, b, :],
                                        c_sb[:, b * 128:(b + 1) * 128],
                                        ident[0:GC, 0:GC])
                cblk = vec.tile([128, MB, GC], F32, tag=tag)
                nc.vector.tensor_copy(cblk[:], tr_ps[:])
                return cblk

            def warm_pe(n):
                wps = ps_s.tile([128, MB, D], F32, tag="mx")
                wv = wps[:].rearrange("p a b -> p (a b)")
                for _ in range(n):
                    nc.tensor.matmul(wv, warm_l[:], warm_r[:],
                                     start=True, stop=True)

            # ================= phase 0: X -> G1 -> allgather =================
            g1_t = vec.tile([128, MB, GC], F16, tag="g_t")
            matvec_G(x_sb[:], xt_sb[:].rearrange("d (b p) -> d b p", p=128),
                     w1t_sb, g1_t)
            nc.sync.dma_start(out=g1_loc[:, :, :], in_=g1_t[:])
            nc.gpsimd.collective_compute(
                "AllGather", ALU.bypass, replica_groups=rg,
                ins=[g1_loc[:, :, :].opt()], outs=[g1_full[:, :, :, :].opt()])
            nc.sync.dma_start(
                out=g1sb[:].rearrange("p (i b) c -> p i b c", i=ncores),
                in_=g1_full[:, :, :, :].rearrange("i p b c -> p i b c"))
            warm_pe(150)

            # ================= pass 1 =================
            c1_ps = ps_c.tile([GC, rows], F32, tag="acc")
            a_pass(g1sb, GC, c1_ps)

            # ================= boundary 1: C1 -> H1 -> G2 =================
            cblk1 = c_to_blocks(c1_ps, "cblk")
            warm_pe(44)
            h1 = vec.tile([128, MB, D], F32, tag="h")
            fs1, un21 = mid_sigma(cblk1, h1)
            ht1 = transpose_blocks(h1, "ht1")
            g2_t = vec.tile([128, MB, GC], F16, tag="g_t")
            matvec_G(h1[:], ht1[:], w2t_sb, g2_t, fs=fs1, un2=un21)
            nc.sync.dma_start(out=g2_loc[:, :, :], in_=g2_t[:])
            nc.gpsimd.collective_compute(
                "AllGather", ALU.bypass, replica_groups=rg,
                ins=[g2_loc[:, :, :].opt()], outs=[g2_full[:, :, :, :].opt()])
            nc.sync.dma_start(
                out=g2sb[:].rearrange("p (i b) c -> p i b c", i=ncores),
                in_=g2_full[:, :, :, :].rearrange("i p b c -> p i b c"))
            warm_pe(60)

            # ================= pass 2 =================
            c2_ps = ps_c.tile([GC, rows], F32, tag="acc")
            a_pass(g2sb, GC, c2_ps)

            # ================= boundary 2: C2 -> H2 -> logits =================
            cblk2 = c_to_blocks(c2_ps, "cblk")
            warm_pe(44)
            h2 = vec.tile([128, MB, D], F32, tag="h")
            fs, un2 = mid_sigma(cblk2, h2)
            ht2 = transpose_blocks(h2, "ht2")
            l_t = vec.tile([128, MB, C], F16, tag="l_t")
            logits_batch(ht2, fs, un2, l_t)
            nc.sync.dma_start(out=l_loc[:, :, :], in_=l_t[:])
            nc.gpsimd.collective_compute(
                "AllGather", ALU.bypass, replica_groups=rg,
                ins=[l_loc[:, :, :].opt()], outs=[l_full[:, :, :, :].opt()])
            nc.sync.dma_start(
                out=lsb[:].rearrange("p (i b) c -> p i b c", i=ncores),
                in_=l_full[:, :, :, :].rearrange("i p b c -> p i b c"))
            warm_pe(40)

            # ================= pass 3: out^T = L^T A^T =================
            o_ps = ps_c.tile([C, rows], F32, tag="o")
            for kt in range(KT):
                for (s0, sl) in NS:
                    nc.tensor.matmul(o_ps[:, s0:s0 + sl], lsb[:, kt, :],
                                     at_all[:, kt, s0:s0 + sl],
                                     start=(kt == 0), stop=(kt == KT - 1))
            o_sb = singles.tile([C, rows], F32, tag="o_sb")
            nc.scalar.activation(o_sb[:], o_ps[:], AF.Copy,
                                 scale=1.0 / SCALE_A)
            nc.sync.dma_start(out=outT[:, :], in_=o_sb[:])

    _split_multiwaits(nc)
    return nc


def _host_inputs(X, A_hat, W1, W2, W_logits, p_ks, N=N_FULL, ncores=NCORES):
    rows = N // ncores
    MB = rows // 128
    KT = N // 128
    f = np.float32
    from ml_dtypes import float8_e4m3fn as f8

    X = np.ascontiguousarray(X, f)
    A_hat = np.ascontiguousarray(A_hat, f)
    AT = np.ascontiguousarray(A_hat.T)
    W1 = np.asarray(W1, f)
    W2 = np.asarray(W2, f)
    WL = np.asarray(W_logits, f)
    PK = np.asarray(p_ks, f)

    x2 = np.sum(PK * PK, axis=-1)                       # |p_k|^2
    a_norm = np.maximum(np.sqrt(np.sum(WL * WL, 0)), 1e-10)
    beta = 1.0 - x2
    xW = np.einsum('kd,dk->k', -PK, WL)
    lam = 2.0 / np.maximum(1.0 - x2, EPS)

    shared = {
        "W1t": np.ascontiguousarray(W1.T, f),
        "W2t": np.ascontiguousarray(W2.T, f),
        "WLb": np.ascontiguousarray(WL * beta[None, :], f),
        "PTn": np.ascontiguousarray(-PK.T, f),
        "cXW": xW.reshape(1, C).astype(f),
        "cBA": (2.0 / (beta * a_norm)).reshape(1, C).astype(f),
        "cLA": (lam * a_norm).reshape(1, C).astype(f),
    }
    in_maps = []
    for i in range(ncores):
        bi = i * rows
        at = np.empty((128, KT, rows), f8)
        for t in range(KT):
            at[:, t, :] = AT[t * 128:(t + 1) * 128, bi:bi + rows] * SCALE_A
        Xl = X[bi:bi + rows]
        m = dict(shared)
        m["At"] = at
        m["Xp"] = np.ascontiguousarray(
            Xl.reshape(MB, 128, D).transpose(1, 0, 2))
        m["Xt"] = np.ascontiguousarray(Xl.T)
        in_maps.append(m)
    return in_maps


_PROGRAM_CACHE = {}


def _get_program(N=N_FULL, ncores=NCORES):
    key = (N, ncores)
    if key not in _PROGRAM_CACHE:
        _PROGRAM_CACHE[key] = build_program(N, ncores)
    return _PROGRAM_CACHE[key]


def run(inputs, trace=False, N=N_FULL, ncores=NCORES):
    nc = _get_program(N, ncores)
    in_maps = _host_inputs(N=N, ncores=ncores, **inputs)
    res = run_bass_kernel_spmd(nc, in_maps, core_ids=list(range(ncores)),
                               trace=trace)
    out = np.concatenate([np.ascontiguousarray(res.results[i]["outT"]).T
                          for i in range(ncores)], axis=0)
    return out.astype(np.float32), res


def kernel(X, A_hat, W1, W2, W_logits, p_ks):
    out, _ = run(dict(X=X, A_hat=A_hat, W1=W1, W2=W2,
                      W_logits=W_logits, p_ks=p_ks))
    return out
